# revision 16
# baseline (speedup 1.0000x reference)
"""GNN message-passing kernel (2-layer GCN-style conv + log_softmax) for
Trainium2, distributed over 8 NeuronCores.

Strategy:
  - Nodes are sharded over the 8 cores (dst-sharding). Each core owns a
    contiguous range of original node ids; within a core, nodes are permuted
    into "A-positions" grouped by padded-degree bucket so the per-node
    segment sums become uniform-stride strided reductions on the vector
    engine.
  - Per conv layer: each core computes Z = (H @ W1.T + b1) @ W2.T + b2 for
    its node shard on the tensor engine (fp16), transposes to row layout,
    and the shards are all-gathered so every core holds the full Z.
  - Messages are fetched with GPSIMD transpose-mode dma_gather (features on
    partitions), in two halves (A = cores 0-3, B = cores 4-7) because gather
    indices are int16. Each half is segment-summed with padded-ELL strided
    tensor_reduce; the B partial (in its own position order) is permuted to
    A-position order with a small SBUF-source gather, then added.
  - log_softmax over features runs on rows after a DMA transpose.
"""

import sys

if "/opt/trn_rl_repo" not in sys.path:
    sys.path.insert(0, "/opt/trn_rl_repo")

import hashlib
from dataclasses import dataclass, field

import numpy as np

import concourse.bacc as bacc
import concourse.bass as bass
import concourse.mybir as mybir
import concourse.tile as tile
from concourse.bass_utils import run_bass_kernel_spmd

# ---------------------------------------------------------------------------
# Tail-drain patch: this walrus build rejects >1 sync wait on one CTRL
# instruction, and bacc's event-semaphore pass does not split waits attached
# to the Tile tail Drain/NoOp. Put each wait on its own NOP instead.
# ---------------------------------------------------------------------------


def _patched_drain_and_barrier(self, tick_clock, wait_clock):
    from concourse.vector_clock import ScopedClock

    nop_inst = self.nc.sync.nop(nofuse=True, hint="tail_drain_waits")
    wait_clock.add_sem_waits(nop_inst.ins, ScopedClock({None: tick_clock.global_clock}))
    si = nop_inst.ins.sync_info
    if si is not None and len(si.on_wait) > 1:
        extra = si.on_wait[1:]
        si.on_wait = si.on_wait[:1]
        for w in extra:
            n2 = self.nc.sync.nop(nofuse=True, hint="tail_drain_waits")
            n2.ins.sync_info = mybir.SyncInfo(on_wait=[w], on_update=[])
    self.nc.sync.drain()
    self.nc.all_engine_barrier()
    assert self.sems is not None
    popped = self.nc._tile_sem_poison_stack.pop()
    assert popped is self._sem_poison
    self.nc.clear_and_free_semaphores(list(self.sems.allocated().values()))
    self.nc.all_engine_barrier()


tile.TileContext._drain_and_barrier = _patched_drain_and_barrier

# ---------------------------------------------------------------------------
# Constants
# ---------------------------------------------------------------------------

P = 8  # cores
F = 128  # feature dim
L_BUCKETS = [1, 2, 3, 4, 6, 8, 12, 16, 24, 32, 48, 64, 96, 128, 192, 256, 384, 512]
CHUNK_COLS = 8192  # max gather columns per dma_gather call

f32 = mybir.dt.float32
f16 = mybir.dt.float16
i16 = mybir.dt.int16


# ---------------------------------------------------------------------------
# Host-side preprocessing
# ---------------------------------------------------------------------------


@dataclass
class HalfStruct:
    """Fixed (shared by all cores) padded-ELL structure for one src-half."""

    n_pos: int  # total positions (sum of bucket capacities)
    pos_of_node: np.ndarray  # [n_nodes] position of each node (within its core)
    chunks: list = field(default_factory=list)
    # chunks: list of (idx_off, nk, runs); runs: list of (loc_col, G, L, pos0)
    total_idx: int = 0  # total index entries (sum of nk)
    flat_start: np.ndarray | None = None  # [n_pos] flat idx-col start per position


def _build_half_struct(deg, node_core, n_cores):
    n_nodes = deg.shape[0]
    lb = np.asarray(L_BUCKETS)
    li = np.searchsorted(lb, np.maximum(deg, 1), side="left")
    assert li.max() < len(lb), "degree exceeds largest bucket"
    nb = len(lb)

    cnt = np.zeros((nb, n_cores), np.int64)
    np.add.at(cnt, (li, node_core), 1)
    cap = cnt.max(axis=1)  # bucket capacity (max over cores)
    pos_off = np.concatenate([[0], np.cumsum(cap)])
    n_pos = int(pos_off[-1])

    # position of each node: pos_off[bucket] + rank within (core, bucket)
    order = np.lexsort((np.arange(n_nodes), li, node_core))
    sorted_key = node_core[order] * nb + li[order]
    starts = np.r_[0, np.flatnonzero(np.diff(sorted_key)) + 1]
    grp_sizes = np.diff(np.r_[starts, len(order)])
    rank_sorted = np.arange(n_nodes) - np.repeat(starts, grp_sizes)
    pos = np.empty(n_nodes, np.int64)
    pos[order] = pos_off[li[order]] + rank_sorted

    st = HalfStruct(n_pos=n_pos, pos_of_node=pos)

    # column layout + chunking (groups never span a chunk)
    flat_start = np.full(n_pos, -1, np.int64)
    idx_off = 0
    cur_runs = []
    cur_cols = 0
    cur_idx_off = 0

    def close_chunk():
        nonlocal cur_runs, cur_cols, idx_off, cur_idx_off
        if cur_cols == 0:
            return
        nk = -(-cur_cols // 128) * 128
        st.chunks.append((cur_idx_off, nk, cur_runs))
        idx_off += nk
        cur_runs = []
        cur_cols = 0
        cur_idx_off = idx_off

    for b in range(nb):
        L = int(lb[b])
        g_left = int(cap[b])
        p0 = int(pos_off[b])
        while g_left > 0:
            gmax = (CHUNK_COLS - cur_cols) // L
            if gmax == 0:
                close_chunk()
                continue
            take = min(g_left, gmax)
            cur_runs.append((cur_cols, take, L, p0))
            flat_start[p0 : p0 + take] = cur_idx_off + cur_cols + np.arange(take) * L
            cur_cols += take * L
            p0 += take
            g_left -= take
            if cur_cols >= CHUNK_COLS:
                close_chunk()
    close_chunk()
    st.total_idx = idx_off
    st.flat_start = flat_start
    return st


def _edge_cols(dst_e, pos_of_node, flat_start):
    """Flat idx-column for each edge: segment start + within-segment rank."""
    order = np.argsort(dst_e, kind="stable")
    sd = dst_e[order]
    starts = np.r_[0, np.flatnonzero(np.diff(sd)) + 1]
    sizes = np.diff(np.r_[starts, len(sd)])
    rank = np.arange(len(sd)) - np.repeat(starts, sizes)
    cols = flat_start[pos_of_node[sd]] + rank
    return order, cols


def _wrap_idx(arr_2d):
    """[P, n] int16 -> [P, 128, n//16] SBUF wrapped+replicated layout."""
    Pn, n = arr_2d.shape
    assert n % 16 == 0
    w = arr_2d.reshape(Pn, n // 16, 16).transpose(0, 2, 1)  # [P,16,n/16]
    return np.tile(w, (1, 8, 1)).astype(np.int16)  # [P,128,n/16]


@dataclass
class Meta:
    n_nodes: int
    npc: int  # original nodes per core
    shard: int  # A-positions per core, multiple of 128 (slot count per core)
    nslot: int
    baseB: int  # row base of the B gather window in ZD
    ztokB: int
    A: HalfStruct
    B: HalfStruct
    bpos: int  # B positions rounded to 128


def preprocess(edge_index, n_nodes):
    assert n_nodes % P == 0
    npc = n_nodes // P
    src = np.asarray(edge_index[0], dtype=np.int64)
    dst = np.asarray(edge_index[1], dtype=np.int64)
    loops = np.arange(n_nodes, dtype=np.int64)
    src = np.concatenate([src, loops])
    dst = np.concatenate([dst, loops])
    node_core = np.arange(n_nodes) // npc
    src_core = src // npc
    dst_core = dst // npc
    in_A = src_core < (P // 2)

    degA = np.bincount(dst[in_A], minlength=n_nodes)
    degB = np.bincount(dst[~in_A], minlength=n_nodes)

    A = _build_half_struct(degA, node_core, P)
    B = _build_half_struct(degB, node_core, P)

    shard = -(-A.n_pos // 128) * 128
    bpos = -(-B.n_pos // 128) * 128
    nslot = P * shard
    # ZD layout: [128 zero rows][nslot slot rows][128 zero rows]
    baseB = max(0, nslot + 256 - 32768)
    assert 4 * shard + 128 <= 32767, shard
    assert nslot + 256 - baseB <= 32768
    ztokB = nslot + 128 - baseB

    meta = Meta(
        n_nodes=n_nodes,
        npc=npc,
        shard=shard,
        nslot=nslot,
        baseB=baseB,
        ztokB=ztokB,
        A=A,
        B=B,
        bpos=bpos,
    )

    # slot of each node (for gather tokens)
    slot = node_core * shard + A.pos_of_node

    # token arrays per core
    idxA = np.zeros((P, A.total_idx), np.int64)  # ZTOK_A = 0 (leading zero rows)
    idxB = np.full((P, B.total_idx), ztokB, np.int64)

    eA = np.flatnonzero(in_A)
    orderA, colsA = _edge_cols(dst[eA], A.pos_of_node, A.flat_start)
    ce = dst_core[eA][orderA]
    idxA[ce, colsA] = slot[src[eA][orderA]] + 128

    eB = np.flatnonzero(~in_A)
    orderB, colsB = _edge_cols(dst[eB], B.pos_of_node, B.flat_start)
    ce = dst_core[eB][orderB]
    idxB[ce, colsB] = slot[src[eB][orderB]] + 128 - baseB
    assert idxA.max() < 32768 and idxB.max() < 32768

    # permutation gather: for each A-position p of core c, the B-position of
    # the node there (zero rank token = bpos for dummy positions)
    idxP = np.full((P, shard), bpos, np.int64)
    nodes = np.arange(n_nodes)
    idxP[node_core, A.pos_of_node] = B.pos_of_node[nodes]
    assert idxP.max() < 32768

    static_inputs = {
        "idxa": _wrap_idx(idxA),
        "idxb": _wrap_idx(idxB),
        "idxp": _wrap_idx(idxP),
    }
    return meta, static_inputs


# ---------------------------------------------------------------------------
# Bass program
# ---------------------------------------------------------------------------


DEBUG_TAPS = ()  # e.g. ("xT", "zT1", "ZD1", "pa1", "pb1", "permb1", "h1")
EXCHANGE = "ag"  # "ag" | "local" (debug: no collective, own shard only)
ENABLE_GATHER = True  # debug: skip dma_gather stages when False


def build_program(meta: Meta, for_sim=False):
    SH = meta.shard
    NB = SH // 128  # row blocks per shard
    BR = meta.bpos // 128

    nc = bacc.Bacc(
        "TRN2",
        target_bir_lowering=False,
        debug=for_sim,
        num_devices=P,
    )

    # I/O
    xs_t = nc.dram_tensor("xs", [SH, F], f32, kind="ExternalInput")
    w1t_t = nc.dram_tensor("w1t", [F, F], f16, kind="ExternalInput")
    w2t_t = nc.dram_tensor("w2t", [F, F], f16, kind="ExternalInput")
    b1_t = nc.dram_tensor("b1c", [F, 1], f32, kind="ExternalInput")
    b2_t = nc.dram_tensor("b2c", [F, 1], f32, kind="ExternalInput")
    idxa_t = nc.dram_tensor("idxa", [128, meta.A.total_idx // 16], i16, kind="ExternalInput")
    idxb_t = nc.dram_tensor("idxb", [128, meta.B.total_idx // 16], i16, kind="ExternalInput")
    idxp_t = nc.dram_tensor("idxp", [128, SH // 16], i16, kind="ExternalInput")
    out_t = nc.dram_tensor("out", [SH, F], f32, kind="ExternalOutput")

    AF = mybir.ActivationFunctionType
    ALU = mybir.AluOpType
    AX = mybir.AxisListType

    dbg_tensors = {}

    def tap(name, ap):
        if name not in DEBUG_TAPS:
            return
        shp = list(ap.shape)
        t = nc.dram_tensor(f"dbg_{name}", shp, ap.dtype, kind="ExternalOutput")
        dbg_tensors[name] = t
        nc.sync.dma_start(t[tuple(slice(None) for _ in shp)], ap)

    with tile.TileContext(nc) as tc:
        with (
            tc.tile_pool(name="dram", bufs=1, space="DRAM") as dpool,
            tc.tile_pool(name="const", bufs=1) as cpool,
            tc.tile_pool(name="hT", bufs=1) as hT_pool,
            tc.tile_pool(name="zT", bufs=1) as zT_pool,
            tc.tile_pool(name="rows", bufs=1) as rows_pool,
            tc.tile_pool(name="brows", bufs=1) as brows_pool,
            tc.tile_pool(name="mm", bufs=3) as mm_pool,
            tc.tile_pool(name="idx", bufs=3) as idx_pool,
            tc.tile_pool(name="gat", bufs=2) as gat_pool,
            tc.tile_pool(name="pa", bufs=1) as pa_pool,
            tc.tile_pool(name="pb", bufs=1) as pb_pool,
            tc.tile_pool(name="pb16", bufs=1) as pb16_pool,
            tc.tile_pool(name="small", bufs=2) as small_pool,
            tc.tile_pool(name="psum", bufs=4, space="PSUM") as psum_pool,
        ):
            # persistent DRAM scratch
            zshard_d = dpool.tile([SH, F], f16)
            ZD = dpool.tile([meta.nslot + 256, F], f16)

            # weights/biases to SBUF
            w1s = cpool.tile([F, F], f16, tag="w1")
            nc.sync.dma_start(w1s[:], w1t_t[:, :])
            w2s = cpool.tile([F, F], f16, tag="w2")
            nc.sync.dma_start(w2s[:], w2t_t[:, :])
            b1s = cpool.tile([F, 1], f32, tag="b1")
            nc.sync.dma_start(b1s[:], b1_t[:, :])
            b2s = cpool.tile([F, 1], f32, tag="b2")
            nc.sync.dma_start(b2s[:], b2_t[:, :])

            # zero the guard rows of ZD (once)
            zrow = cpool.tile([128, F], f16, tag="zrow")
            nc.vector.memset(zrow[:], 0.0)
            nc.sync.dma_start(ZD[0:128, :], zrow[:])
            nc.sync.dma_start(ZD[meta.nslot + 128 : meta.nslot + 256, :], zrow[:])

            def linear_block(hT):
                """hT fp16 [128, SH] (features on partitions) -> zT fp16."""
                zT = zT_pool.tile([128, SH], f16, tag="zT")
                for j in range(0, SH, 512):
                    n = min(512, SH - j)
                    ps1 = psum_pool.tile([128, 512], f32, tag="ps")
                    nc.tensor.matmul(
                        ps1[:, :n], w1s[:], hT[:, j : j + n], start=True, stop=True
                    )
                    h1c = mm_pool.tile([128, 512], f16, tag="h1c")
                    nc.scalar.activation(h1c[:, :n], ps1[:, :n], AF.Identity, bias=b1s[:])
                    ps2 = psum_pool.tile([128, 512], f32, tag="ps")
                    nc.tensor.matmul(
                        ps2[:, :n], w2s[:], h1c[:, :n], start=True, stop=True
                    )
                    nc.scalar.activation(
                        zT[:, j : j + n], ps2[:, :n], AF.Identity, bias=b2s[:]
                    )
                return zT

            def gather_reduce(st: HalfStruct, idx_dram, in_ap, partial, n_pos):
                """Run all gather chunks of one half and strided-reduce into
                `partial` (fp32 [128, >=n_pos])."""
                if not ENABLE_GATHER:
                    nc.vector.memset(partial[:, :n_pos], 0.0)
                    return
                for idx_off, nk, runs in st.chunks:
                    it = idx_pool.tile([128, CHUNK_COLS // 16], i16, tag="idx")
                    nc.sync.dma_start(
                        it[:, : nk // 16],
                        idx_dram[:, idx_off // 16 : (idx_off + nk) // 16],
                    )
                    gt = gat_pool.tile([128, 1, CHUNK_COLS], f16, tag="gat")
                    nc.gpsimd.dma_gather(
                        gt[:, :, :nk],
                        in_ap,
                        it[:, : nk // 16],
                        num_idxs=nk,
                        num_idxs_reg=nk,
                        elem_size=F,
                        transpose=True,
                        single_packet=False,
                    )
                    for loc, G, L, pos0 in runs:
                        view = gt[:, 0, loc : loc + G * L].rearrange(
                            "p (g l) -> p g l", l=L
                        )
                        nc.vector.tensor_reduce(
                            partial[:, pos0 : pos0 + G], view, axis=AX.X, op=ALU.add
                        )

            def conv_layer(hT, layer):
                zT = linear_block(hT)
                tap(f"zT{layer}", zT[:, :])
                # transpose to row layout and push the shard out
                z_rows = rows_pool.tile([128, NB, 128], f16, tag="rows")
                nc.sync.dma_start_transpose(z_rows[:], zT[:])
                nc.sync.dma_start(
                    zshard_d.rearrange("(b p) f -> p b f", p=128), z_rows[:]
                )
                if EXCHANGE == "ag":
                    nc.gpsimd.collective_compute(
                        "AllGather",
                        ALU.bypass,
                        replica_groups=[list(range(P))],
                        ins=[zshard_d[:, :].opt()],
                        outs=[ZD[128 : 128 + meta.nslot, :].opt()],
                    )
                else:  # debug: only own shard, placed at slot 0
                    nc.sync.dma_start(ZD[128 : 128 + SH, :], zshard_d[:, :])
                tap(f"ZD{layer}", ZD[:, :])

                pa = pa_pool.tile([128, SH], f32, tag="pa")
                if meta.A.n_pos < SH:
                    nc.vector.memset(pa[:, meta.A.n_pos :], 0.0)
                gather_reduce(
                    meta.A, idxa_t, ZD[0 : 4 * SH + 128, :], pa, meta.A.n_pos
                )
                tap(f"pa{layer}", pa[:, :])

                pb = pb_pool.tile([128, meta.bpos], f32, tag="pb")
                if meta.B.n_pos < meta.bpos:
                    nc.vector.memset(pb[:, meta.B.n_pos :], 0.0)
                gather_reduce(
                    meta.B,
                    idxb_t,
                    ZD[meta.baseB : meta.nslot + 256, :],
                    pb,
                    meta.B.n_pos,
                )
                tap(f"pb{layer}", pb[:, :])

                if not ENABLE_GATHER:
                    return pa
                # permute B-partial into A-position order and add
                pb16 = pb16_pool.tile([128, meta.bpos], f16, tag="pb16")
                nc.vector.tensor_copy(pb16[:], pb[:])
                b_rows = brows_pool.tile([128, BR + 1, 128], f16, tag="brows")
                nc.sync.dma_start_transpose(b_rows[:, 0:BR, :], pb16[:])
                nc.vector.memset(b_rows[:, BR, :], 0.0)

                itp = idx_pool.tile([128, SH // 16], i16, tag="idxp")
                nc.sync.dma_start(itp[:, : SH // 16], idxp_t[:, :])
                permb = gat_pool.tile([128, 1, SH], f16, tag="permb")
                for j in range(0, SH, CHUNK_COLS):
                    nj = min(CHUNK_COLS, SH - j)
                    nc.gpsimd.dma_gather(
                        permb[:, :, j : j + nj],
                        b_rows.rearrange("p r f -> p (r f)"),
                        itp[:, j // 16 : (j + nj) // 16],
                        num_idxs=nj,
                        num_idxs_reg=nj,
                        elem_size=F,
                        transpose=True,
                        single_packet=False,
                        sbuf_tokens_per_rank=128,
                        sbuf_free_dim_per_rank=256,
                    )
                tap(f"permb{layer}", permb[:, 0, :SH])
                nc.vector.tensor_tensor(
                    pa[:], pa[:], permb[:, 0, :SH], op=ALU.add
                )
                tap(f"h{layer}", pa[:, :])
                return pa  # fp32 [128, SH] = H'.T

            # ---- layer 1 ----
            x_rows = rows_pool.tile([128, NB, 128], f16, tag="rows")
            nc.gpsimd.dma_start(
                x_rows[:], xs_t.rearrange("(b p) f -> p b f", p=128)
            )  # fp32 -> fp16 cast in SWDGE
            xT = hT_pool.tile([128, SH], f16, tag="hT")
            nc.sync.dma_start_transpose(
                xT.rearrange("p (b n) -> p b n", n=128), x_rows[:]
            )
            tap("xT", xT[:, :])
            h1 = conv_layer(xT, 1)

            # ---- layer 2 ----
            h1_16 = hT_pool.tile([128, SH], f16, tag="hT")
            nc.vector.tensor_copy(h1_16[:], h1[:])
            h2 = conv_layer(h1_16, 2)

            # ---- log_softmax over features ----
            h2_16 = zT_pool.tile([128, SH], f16, tag="zT")
            nc.vector.tensor_copy(h2_16[:], h2[:])
            h2_rows = rows_pool.tile([128, NB, 128], f16, tag="rows")
            nc.sync.dma_start_transpose(h2_rows[:], h2_16[:])

            exp32 = pb_pool.tile([128, SH], f32, tag="pb")
            nc.scalar.activation(
                exp32[:], h2_rows.rearrange("p b f -> p (b f)"), AF.Exp
            )
            s = small_pool.tile([128, NB], f32, tag="s")
            nc.vector.tensor_reduce(
                s[:], exp32.rearrange("p (b f) -> p b f", f=128), axis=AX.X, op=ALU.add
            )
            lse = small_pool.tile([128, NB], f32, tag="lse")
            nc.scalar.activation(lse[:], s[:], AF.Ln)

            out_sb = pa_pool.tile([128, NB, 128], f32, tag="pa")
            for b in range(NB):
                nc.vector.tensor_scalar(
                    out_sb[:, b, :],
                    h2_rows[:, b, :],
                    lse[:, b : b + 1],
                    None,
                    ALU.subtract,
                )
            nc.sync.dma_start(out_t.rearrange("(b p) f -> p b f", p=128), out_sb[:])

    nc.finalize()
    return nc


# ---------------------------------------------------------------------------
# Entry point
# ---------------------------------------------------------------------------

_CACHE = {}


def _get_compiled(edge_index, n_nodes):
    key = hashlib.sha1(np.ascontiguousarray(edge_index).tobytes()).hexdigest()
    if key not in _CACHE:
        meta, static_inputs = preprocess(edge_index, n_nodes)
        nc = build_program(meta)
        _CACHE[key] = (meta, static_inputs, nc)
    return _CACHE[key]


def make_in_maps(meta, static_inputs, x, w1, b1, w2, b2):
    n_nodes = meta.n_nodes
    SH = meta.shard
    node_core = np.arange(n_nodes) // meta.npc
    pos = meta.A.pos_of_node

    w1t = np.ascontiguousarray(w1.T).astype(np.float16)
    w2t = np.ascontiguousarray(w2.T).astype(np.float16)
    b1c = np.ascontiguousarray(b1.reshape(F, 1)).astype(np.float32)
    b2c = np.ascontiguousarray(b2.reshape(F, 1)).astype(np.float32)

    in_maps = []
    for c in range(P):
        xs = np.zeros((SH, F), np.float32)
        nodes = np.arange(c * meta.npc, (c + 1) * meta.npc)
        xs[pos[nodes]] = x[nodes]
        in_maps.append(
            {
                "xs": xs,
                "w1t": w1t,
                "w2t": w2t,
                "b1c": b1c,
                "b2c": b2c,
                "idxa": static_inputs["idxa"][c],
                "idxb": static_inputs["idxb"][c],
                "idxp": static_inputs["idxp"][c],
            }
        )
    return in_maps


def assemble_output(meta, results, n_nodes):
    y = np.empty((n_nodes, F), np.float32)
    pos = meta.A.pos_of_node
    for c in range(P):
        nodes = np.arange(c * meta.npc, (c + 1) * meta.npc)
        y[nodes] = results[c]["out"][pos[nodes]]
    return y


def kernel(x, edge_index, w1, b1, w2, b2):
    x = np.asarray(x, np.float32)
    edge_index = np.asarray(edge_index)
    n_nodes = x.shape[0]
    meta, static_inputs, nc = _get_compiled(edge_index, n_nodes)
    in_maps = make_in_maps(meta, static_inputs, x, w1, b1, w2, b2)
    res = run_bass_kernel_spmd(nc, in_maps, core_ids=list(range(P)))
    return assemble_output(meta, res.results, n_nodes)
